# revision 1
# baseline (speedup 1.0000x reference)
"""PacConv2d (BlockPAC) Trainium2 kernel.

nn_BlockPAC: guide-adaptive 3x3 convolution (PAC) + bias + relu.
  kernel[b,p,h,w] = exp(-0.5 * sum_cg (guide_tap_p - guide_center)^2)
  out[b,o,h,w]    = relu(bias[o] + sum_{c,p} x_tap_p[b,c,h,w] * kernel[b,p,h,w]
                                            * weight[o,c,p])

Sharding: data-parallel over batch B=8 across the 8 NeuronCores (one sample
per core). No collectives.

Host side does layout only (zero-pad + im2col tap stacking + bf16 cast);
all arithmetic (diff, square, sum over guide channels, exp, the adaptive
multiply, the weight contraction, bias, relu) runs on device.

Per-core device pipeline (sample = x(64,128,128), guide(16,128,128)),
8 row-blocks of 16 output rows:
  * DMA in: padded fp32 x slice (center-tap path), 4 pre-stacked x tap
    tiles (128 = 8 taps x 16 chans of group g), guide tap/center stacks.
  * diff = gs - gc (DVE), sq = diff^2 (ACT Square).
  * D-matmul: lhsT(128,128) = block(-0.5) x sq -> PSUM: computes
    -0.5*sum_cg AND replicates each tap's D across 16 partitions.
  * E = exp(D) (ACT, PSUM->SBUF, bf16).
  * y[g] = xstk[g] * E (DVE tensor_mul, bf16 2x mode).
  * out PSUM += sum_g W_g^T y_g   (4 bf16 matmuls, K=128)
             +  Wc^T x_center     (1 fp32 matmul, K=64; exact center path)
  * relu(out + bias) in one ACT op, DMA out.

Precision: with randn guides the non-center kernel weights are ~exp(-16),
so the output is dominated by the center tap, which stays fully fp32; bf16
on the tap/guide path contributes ~4e-5 relative error overall.
"""

import os
import sys

import numpy as np

sys.path.insert(0, "/opt/trn_rl_repo")

import ml_dtypes

from concourse import bass, mybir, tile
from concourse.bass_utils import run_bass_kernel_spmd

# ---------------------------------------------------------------- constants
B, CIN, COUT, CG, H, W = 8, 64, 64, 16, 128, 128
KS, PAD = 3, 1
HP, WP = H + 2 * PAD, W + 2 * PAD  # 130, 130
NCORES = 8

R = 16                      # output rows per block
NBLK = H // R               # 8 blocks
RH = R + 2                  # padded rows per block (halo)
HGRP = 8                    # rows per psum group (2 chunks of 4)
CH = 4                      # output rows per matmul chunk (N = 4*128 = 512)

# non-center taps p=3i+j, p != 4, in reference order
TAPS = [(p // 3, p % 3) for p in range(9) if p != 4]
NT = len(TAPS)              # 8
CTR_I, CTR_J = 1, 1

USE_BF16 = os.environ.get("PAC_BF16", "1") == "1"
F32 = mybir.dt.float32
BF = mybir.dt.bfloat16 if USE_BF16 else F32
NPBF = ml_dtypes.bfloat16 if USE_BF16 else np.float32

_cache = {}


# ---------------------------------------------------------------- bass build
def _build_nc():
    nc = bass.Bass(
        "TRN2",
        target_bir_lowering=False,
        debug=False,
        enable_asserts=False,
        num_devices=NCORES,
    )

    xp_d = nc.dram_tensor("xp", [CIN, HP, WP], F32, kind="ExternalInput").ap()
    xstk_d = nc.dram_tensor("xstk", [4 * 128, H, W], BF, kind="ExternalInput").ap()
    gs_d = nc.dram_tensor("gs", [128, H, W], BF, kind="ExternalInput").ap()
    gc_d = nc.dram_tensor("gc", [128, H, W], BF, kind="ExternalInput").ap()
    wstk_d = nc.dram_tensor("wstk", [4 * 128, COUT], BF, kind="ExternalInput").ap()
    wctr_d = nc.dram_tensor("wctr", [CIN, COUT], F32, kind="ExternalInput").ap()
    lhsd_d = nc.dram_tensor("lhsd", [128, 128], BF, kind="ExternalInput").ap()
    bias_d = nc.dram_tensor("bias", [COUT, 1], F32, kind="ExternalInput").ap()
    out_d = nc.dram_tensor("out", [COUT, H, W], F32, kind="ExternalOutput").ap()

    with tile.TileContext(nc) as tc:
        import contextlib

        with contextlib.ExitStack() as ctx:
            cst = ctx.enter_context(tc.tile_pool(name="cst", bufs=1))
            blk = ctx.enter_context(tc.tile_pool(name="blk", bufs=3))
            cnk = ctx.enter_context(tc.tile_pool(name="cnk", bufs=3))
            psd = ctx.enter_context(tc.tile_pool(name="psd", bufs=2, space="PSUM"))
            pso = ctx.enter_context(tc.tile_pool(name="pso", bufs=2, space="PSUM"))

            # constants
            wstk_t = []
            for g in range(4):
                wt = cst.tile([128, COUT], BF, name=f"wstk{g}")
                nc.sync.dma_start(wt[:], wstk_d[128 * g : 128 * (g + 1), :])
                wstk_t.append(wt)
            wctr_t = cst.tile([CIN, COUT], F32, name="wctr")
            nc.sync.dma_start(wctr_t[:], wctr_d[:])
            lhsd_t = cst.tile([128, 128], BF, name="lhsd")
            nc.sync.dma_start(lhsd_t[:], lhsd_d[:])
            bias_t = cst.tile([COUT, 1], F32, name="bias")
            nc.sync.dma_start(bias_t[:], bias_d[:])

            for b in range(NBLK):
                r0 = R * b  # first output row of block == first padded row

                xp32 = blk.tile([CIN, RH, WP], F32, name="xp32")
                nc.sync.dma_start(xp32[:], xp_d[:, r0 : r0 + RH, :])

                xstk = []
                for g in range(4):
                    st = blk.tile([128, R, W], BF, name=f"xstk{g}", bufs=4)
                    nc.scalar.dma_start(
                        st[:], xstk_d[128 * g : 128 * (g + 1), r0 : r0 + R, :]
                    )
                    xstk.append(st)

                gstks = blk.tile([128, R, W], BF, name="gstks")
                nc.sync.dma_start(gstks[:], gs_d[:, r0 : r0 + R, :])
                gstkc = blk.tile([128, R, W], BF, name="gstkc")
                nc.sync.dma_start(gstkc[:], gc_d[:, r0 : r0 + R, :])

                diff = blk.tile([128, R, W], BF, name="diff")
                nc.vector.tensor_sub(diff[:], gstks[:], gstkc[:])
                sq = blk.tile([128, R, W], BF, name="sq")
                nc.scalar.square(sq[:], diff[:])

                e8 = blk.tile([128, R, W], BF, name="e8")

                for h in range(R // HGRP):  # 2 psum groups of 8 rows
                    hr = HGRP * h
                    dps = psd.tile([128, HGRP, W], F32, name="dps")
                    for q in range(HGRP // CH):
                        nc.tensor.matmul(
                            dps[:, CH * q : CH * (q + 1), :],
                            lhsd_t[:],
                            sq[:, hr + CH * q : hr + CH * (q + 1), :],
                            start=True,
                            stop=True,
                        )
                    nc.scalar.activation(
                        e8[:, hr : hr + HGRP, :],
                        dps[:],
                        mybir.ActivationFunctionType.Exp,
                    )

                    ops = pso.tile([COUT, HGRP, W], F32, name="ops")
                    ys = []
                    for g in range(4):
                        yt = cnk.tile([128, HGRP, W], BF, name=f"y{g}")
                        nc.vector.tensor_mul(
                            yt[:],
                            xstk[g][:, hr : hr + HGRP, :],
                            e8[:, hr : hr + HGRP, :],
                        )
                        ys.append(yt)
                    for q in range(HGRP // CH):
                        r = hr + CH * q
                        for g in range(4):
                            nc.tensor.matmul(
                                ops[:, CH * q : CH * (q + 1), :],
                                wstk_t[g][:],
                                ys[g][:, CH * q : CH * (q + 1), :],
                                start=(g == 0),
                                stop=False,
                            )
                        nc.tensor.matmul(
                            ops[:, CH * q : CH * (q + 1), :],
                            wctr_t[:],
                            xp32[:, CTR_I + r : CTR_I + r + CH, CTR_J : CTR_J + W],
                            start=False,
                            stop=True,
                        )

                    osb = cnk.tile([COUT, HGRP, W], F32, name="osb")
                    nc.scalar.activation(
                        osb[:],
                        ops[:],
                        mybir.ActivationFunctionType.Relu,
                        bias=bias_t[:],
                    )
                    nc.sync.dma_start(out_d[:, r0 + hr : r0 + hr + HGRP, :], osb[:])

    _split_waits(nc)
    return nc


_SKIP_SPLIT = {"InstCall", "InstUnconditionalBranch", "InstEventSemaphore"}


def _split_waits(nc):
    """Walrus's PSEUDO_DMA_DIRECT2D (and friends) carry a single sync-wait
    slot; Tile can attach several. Peel extra waits onto single-wait
    EventSemaphore instructions on the same engine immediately before the
    instruction (classic raw-bass wait-then-issue pattern)."""
    nopctr = [0]
    scratch_id = max(int(k) for k in nc.m.ant_sem_names) + 1
    nc.m.ant_sem_names[str(scratch_id)] = ["waitnop_scratch"]

    def mk_nop(engine, wait):
        nopctr[0] += 1
        nop = mybir.InstEventSemaphore(
            name=f"I-waitnop-{nopctr[0]}", ins=[], outs=[]
        )
        nop.engine = engine
        upd = mybir.SyncUpdate(
            sync_type="semaphore",
            id=scratch_id,
            ant_name="waitnop_scratch",
            update_mode="sem-add-imm",
            update_value=0,
            update_reg=None,
        )
        nop.sync_info = mybir.SyncInfo(on_wait=[wait], on_update=[upd])
        return nop

    for f in nc.m.functions:
        for blk in f.blocks:
            out = []
            for inst in blk.instructions:
                si = inst.sync_info
                if (
                    si is not None
                    and si.on_wait
                    and len(si.on_wait) > 1
                    and type(inst).__name__ not in _SKIP_SPLIT
                ):
                    waits = list(si.on_wait)
                    for w in waits[:-1]:
                        out.append(mk_nop(inst.engine, w))
                    inst.sync_info = mybir.SyncInfo(
                        on_wait=[waits[-1]], on_update=list(si.on_update)
                    )
                out.append(inst)
            blk.instructions[:] = out


def _get_nc():
    if "nc" not in _cache:
        _cache["nc"] = _build_nc()
    return _cache["nc"]


# ---------------------------------------------------------------- host side
def _prep_inputs(x, guide, weight, bias):
    x = np.asarray(x, dtype=np.float32)
    guide = np.asarray(guide, dtype=np.float32)
    weight = np.asarray(weight, dtype=np.float32)
    bias = np.asarray(bias, dtype=np.float32)

    xp = np.pad(x, ((0, 0), (0, 0), (PAD, PAD), (PAD, PAD)))
    gp = np.pad(guide, ((0, 0), (0, 0), (PAD, PAD), (PAD, PAD))).astype(NPBF)
    xpb = xp.astype(NPBF)

    # pre-stacked im2col tap tensors (pure layout, no arithmetic)
    xstk = np.empty((B, 4 * 128, H, W), dtype=NPBF)
    gs = np.empty((B, 128, H, W), dtype=NPBF)
    gc = np.empty((B, 128, H, W), dtype=NPBF)
    for t, (ti, tj) in enumerate(TAPS):
        for g in range(4):
            xstk[:, 128 * g + 16 * t : 128 * g + 16 * t + 16] = xpb[
                :, 16 * g : 16 * g + 16, ti : ti + H, tj : tj + W
            ]
        gs[:, 16 * t : 16 * t + 16] = gp[:, :, ti : ti + H, tj : tj + W]
        gc[:, 16 * t : 16 * t + 16] = gp[
            :, :, CTR_I : CTR_I + H, CTR_J : CTR_J + W
        ]

    # wstk[g][16*t + i, o] = weight[o, 16g+i, ti, tj]
    wstk = np.zeros((4 * 128, COUT), dtype=np.float32)
    for g in range(4):
        for t, (ti, tj) in enumerate(TAPS):
            wstk[128 * g + 16 * t : 128 * g + 16 * t + 16, :] = weight[
                :, 16 * g : 16 * g + 16, ti, tj
            ].T
    wstk = wstk.astype(NPBF)
    wctr = np.ascontiguousarray(weight[:, :, CTR_I, CTR_J].T)  # (CIN, COUT) fp32

    lhsd = np.zeros((128, 128), dtype=np.float32)
    for t in range(NT):
        lhsd[16 * t : 16 * t + 16, 16 * t : 16 * t + 16] = -0.5
    lhsd = lhsd.astype(NPBF)

    bias2 = bias.reshape(COUT, 1).astype(np.float32)

    in_maps = []
    for i in range(NCORES):
        in_maps.append(
            {
                "xp": np.ascontiguousarray(xp[i]),
                "xstk": np.ascontiguousarray(xstk[i]),
                "gs": np.ascontiguousarray(gs[i]),
                "gc": np.ascontiguousarray(gc[i]),
                "wstk": wstk,
                "wctr": wctr,
                "lhsd": lhsd,
                "bias": bias2,
            }
        )
    return in_maps


def _run(in_maps, trace=False, **kw):
    nc = _get_nc()
    last = None
    for attempt in range(3):
        try:
            res = run_bass_kernel_spmd(
                nc, in_maps, list(range(NCORES)), trace=trace, **kw
            )
            break
        except Exception as e:  # wedged device: wait and retry
            last = e
            import time as _t

            _t.sleep(20 * (attempt + 1))
    else:
        raise last
    out = np.stack([res.results[i]["out"] for i in range(NCORES)], axis=0)
    return out.astype(np.float32), res


def kernel(x, guide, weight, bias):
    in_maps = _prep_inputs(x, guide, weight, bias)
    out, _ = _run(in_maps)
    return out



# revision 2
# speedup vs baseline: 2.0631x; 2.0631x over previous
"""PacConv2d (BlockPAC) Trainium2 kernel.

nn_BlockPAC: guide-adaptive 3x3 convolution (PAC) + bias + relu.
  kernel[b,p,h,w] = exp(-0.5 * sum_cg (guide_tap_p - guide_center)^2)
  out[b,o,h,w]    = relu(bias[o] + sum_{c,p} x_tap_p[b,c,h,w] * kernel[b,p,h,w]
                                            * weight[o,c,p])

Sharding: data-parallel over batch B=8 across the 8 NeuronCores (one sample
per core). No collectives.

Host side does layout only (zero-pad + im2col tap stacking + bf16 cast);
all arithmetic (diff, square, sum over guide channels, exp, the adaptive
multiply, the weight contraction, bias, relu) runs on device.

Per-core device pipeline (sample = x(64,128,128), guide(16,128,128)),
8 row-blocks of 16 output rows:
  * DMA in: padded fp32 x slice (center-tap path), 4 pre-stacked x tap
    tiles (128 = 8 taps x 16 chans of group g), guide tap/center stacks.
  * diff = gs - gc (DVE), sq = diff^2 (ACT Square).
  * D-matmul: lhsT(128,128) = block(-0.5) x sq -> PSUM: computes
    -0.5*sum_cg AND replicates each tap's D across 16 partitions.
  * E = exp(D) (ACT, PSUM->SBUF, bf16).
  * y[g] = xstk[g] * E (DVE tensor_mul, bf16 2x mode).
  * out PSUM += sum_g W_g^T y_g   (4 bf16 matmuls, K=128)
             +  Wc^T x_center     (1 fp32 matmul, K=64; exact center path)
  * relu(out + bias) in one ACT op, DMA out.

Precision: with randn guides the non-center kernel weights are ~exp(-16),
so the output is dominated by the center tap, which stays fully fp32; bf16
on the tap/guide path contributes ~4e-5 relative error overall.
"""

import os
import sys

import numpy as np

sys.path.insert(0, "/opt/trn_rl_repo")

import ml_dtypes

from concourse import bass, mybir, tile
from concourse.bass_utils import run_bass_kernel_spmd

# ---------------------------------------------------------------- constants
B, CIN, COUT, CG, H, W = 8, 64, 64, 16, 128, 128
KS, PAD = 3, 1
HP, WP = H + 2 * PAD, W + 2 * PAD  # 130, 130
NCORES = 8

R = 16                      # output rows per block
NBLK = H // R               # 8 blocks
RH = R + 2                  # padded rows per block (halo)
HGRP = 8                    # rows per psum group (2 chunks of 4)
CH = 4                      # output rows per matmul chunk (N = 4*128 = 512)

# non-center taps p=3i+j, p != 4, in reference order
TAPS = [(p // 3, p % 3) for p in range(9) if p != 4]
NT = len(TAPS)              # 8
CTR_I, CTR_J = 1, 1

USE_BF16 = os.environ.get("PAC_BF16", "1") == "1"
UNROLL = int(os.environ.get("PAC_UNROLL", "1"))
F32 = mybir.dt.float32
BF = mybir.dt.bfloat16 if USE_BF16 else F32
NPBF = ml_dtypes.bfloat16 if USE_BF16 else np.float32

_cache = {}


# ---------------------------------------------------------------- bass build
def _build_nc():
    nc = bass.Bass(
        "TRN2",
        target_bir_lowering=False,
        debug=False,
        enable_asserts=False,
        num_devices=NCORES,
    )

    xp_d = nc.dram_tensor("xp", [CIN, HP, WP], F32, kind="ExternalInput").ap()
    xstk_d = nc.dram_tensor("xstk", [4 * 128, H, W], BF, kind="ExternalInput").ap()
    gs_d = nc.dram_tensor("gs", [128, H, W], BF, kind="ExternalInput").ap()
    gc_d = nc.dram_tensor("gc", [128, H, W], BF, kind="ExternalInput").ap()
    wstk_d = nc.dram_tensor("wstk", [4 * 128, COUT], BF, kind="ExternalInput").ap()
    wctr_d = nc.dram_tensor("wctr", [CIN, COUT], F32, kind="ExternalInput").ap()
    lhsd_d = nc.dram_tensor("lhsd", [128, 128], BF, kind="ExternalInput").ap()
    bias_d = nc.dram_tensor("bias", [COUT, 1], F32, kind="ExternalInput").ap()
    out_d = nc.dram_tensor("out", [COUT, H, W], F32, kind="ExternalOutput").ap()

    with tile.TileContext(nc) as tc:
        import contextlib

        with contextlib.ExitStack() as ctx:
            cst = ctx.enter_context(tc.tile_pool(name="cst", bufs=1))
            blk = ctx.enter_context(tc.tile_pool(name="blk", bufs=3))
            cnk = ctx.enter_context(tc.tile_pool(name="cnk", bufs=3))
            psd = ctx.enter_context(tc.tile_pool(name="psd", bufs=2, space="PSUM"))
            pso = ctx.enter_context(tc.tile_pool(name="pso", bufs=2, space="PSUM"))

            # constants
            wstk_t = []
            for g in range(4):
                wt = cst.tile([128, COUT], BF, name=f"wstk{g}")
                nc.sync.dma_start(wt[:], wstk_d[128 * g : 128 * (g + 1), :])
                wstk_t.append(wt)
            wctr_t = cst.tile([CIN, COUT], F32, name="wctr")
            nc.sync.dma_start(wctr_t[:], wctr_d[:])
            lhsd_t = cst.tile([128, 128], BF, name="lhsd")
            nc.sync.dma_start(lhsd_t[:], lhsd_d[:])
            bias_t = cst.tile([COUT, 1], F32, name="bias")
            nc.sync.dma_start(bias_t[:], bias_d[:])

            for b in range(NBLK * UNROLL):
                b = b % NBLK
                r0 = R * b  # first output row of block == first padded row

                xp32 = blk.tile([CIN, RH, WP], F32, name="xp32")
                nc.sync.dma_start(xp32[:], xp_d[:, r0 : r0 + RH, :])

                xstk = []
                for g in range(4):
                    st = blk.tile([128, R, W], BF, name=f"xstk{g}", bufs=4)
                    nc.scalar.dma_start(
                        st[:], xstk_d[128 * g : 128 * (g + 1), r0 : r0 + R, :]
                    )
                    xstk.append(st)

                gstks = blk.tile([128, R, W], BF, name="gstks")
                nc.sync.dma_start(gstks[:], gs_d[:, r0 : r0 + R, :])
                gstkc = blk.tile([128, R, W], BF, name="gstkc")
                nc.sync.dma_start(gstkc[:], gc_d[:, r0 : r0 + R, :])

                diff = blk.tile([128, R, W], BF, name="diff")
                nc.vector.tensor_sub(diff[:], gstks[:], gstkc[:])
                sq = blk.tile([128, R, W], BF, name="sq")
                nc.scalar.square(sq[:], diff[:])

                e8 = blk.tile([128, R, W], BF, name="e8")

                for h in range(R // HGRP):  # 2 psum groups of 8 rows
                    hr = HGRP * h
                    dps = psd.tile([128, HGRP, W], F32, name="dps")
                    for q in range(HGRP // CH):
                        nc.tensor.matmul(
                            dps[:, CH * q : CH * (q + 1), :],
                            lhsd_t[:],
                            sq[:, hr + CH * q : hr + CH * (q + 1), :],
                            start=True,
                            stop=True,
                        )
                    nc.scalar.activation(
                        e8[:, hr : hr + HGRP, :],
                        dps[:],
                        mybir.ActivationFunctionType.Exp,
                    )

                    ops = pso.tile([COUT, HGRP, W], F32, name="ops")
                    ys = []
                    for g in range(4):
                        yt = cnk.tile([128, HGRP, W], BF, name=f"y{g}")
                        nc.vector.tensor_mul(
                            yt[:],
                            xstk[g][:, hr : hr + HGRP, :],
                            e8[:, hr : hr + HGRP, :],
                        )
                        ys.append(yt)
                    for q in range(HGRP // CH):
                        r = hr + CH * q
                        for g in range(4):
                            nc.tensor.matmul(
                                ops[:, CH * q : CH * (q + 1), :],
                                wstk_t[g][:],
                                ys[g][:, CH * q : CH * (q + 1), :],
                                start=(g == 0),
                                stop=False,
                            )
                        nc.tensor.matmul(
                            ops[:, CH * q : CH * (q + 1), :],
                            wctr_t[:],
                            xp32[:, CTR_I + r : CTR_I + r + CH, CTR_J : CTR_J + W],
                            start=False,
                            stop=True,
                        )

                    osb = cnk.tile([COUT, HGRP, W], F32, name="osb")
                    nc.scalar.activation(
                        osb[:],
                        ops[:],
                        mybir.ActivationFunctionType.Relu,
                        bias=bias_t[:],
                    )
                    nc.sync.dma_start(out_d[:, r0 + hr : r0 + hr + HGRP, :], osb[:])

    _split_waits(nc)
    return nc


_SKIP_SPLIT = {"InstCall", "InstUnconditionalBranch", "InstEventSemaphore"}


def _split_waits(nc):
    """Walrus's PSEUDO_DMA_DIRECT2D (and friends) carry a single sync-wait
    slot; Tile can attach several. Peel extra waits onto single-wait
    EventSemaphore instructions on the same engine immediately before the
    instruction (classic raw-bass wait-then-issue pattern)."""
    nopctr = [0]
    scratch_id = max(int(k) for k in nc.m.ant_sem_names) + 1
    nc.m.ant_sem_names[str(scratch_id)] = ["waitnop_scratch"]

    def mk_nop(engine, wait):
        nopctr[0] += 1
        nop = mybir.InstEventSemaphore(
            name=f"I-waitnop-{nopctr[0]}", ins=[], outs=[]
        )
        nop.engine = engine
        upd = mybir.SyncUpdate(
            sync_type="semaphore",
            id=scratch_id,
            ant_name="waitnop_scratch",
            update_mode="sem-add-imm",
            update_value=0,
            update_reg=None,
        )
        nop.sync_info = mybir.SyncInfo(on_wait=[wait], on_update=[upd])
        return nop

    for f in nc.m.functions:
        for blk in f.blocks:
            out = []
            for inst in blk.instructions:
                si = inst.sync_info
                if (
                    si is not None
                    and si.on_wait
                    and len(si.on_wait) > 1
                    and type(inst).__name__ not in _SKIP_SPLIT
                ):
                    waits = list(si.on_wait)
                    for w in waits[:-1]:
                        out.append(mk_nop(inst.engine, w))
                    inst.sync_info = mybir.SyncInfo(
                        on_wait=[waits[-1]], on_update=list(si.on_update)
                    )
                out.append(inst)
            blk.instructions[:] = out


def _get_nc():
    if "nc" not in _cache:
        _cache["nc"] = _build_nc()
    return _cache["nc"]


# ---------------------------------------------------------------- host side
def _prep_inputs(x, guide, weight, bias):
    x = np.asarray(x, dtype=np.float32)
    guide = np.asarray(guide, dtype=np.float32)
    weight = np.asarray(weight, dtype=np.float32)
    bias = np.asarray(bias, dtype=np.float32)

    xp = np.pad(x, ((0, 0), (0, 0), (PAD, PAD), (PAD, PAD)))
    gp = np.pad(guide, ((0, 0), (0, 0), (PAD, PAD), (PAD, PAD))).astype(NPBF)
    xpb = xp.astype(NPBF)

    # pre-stacked im2col tap tensors (pure layout, no arithmetic)
    xstk = np.empty((B, 4 * 128, H, W), dtype=NPBF)
    gs = np.empty((B, 128, H, W), dtype=NPBF)
    gc = np.empty((B, 128, H, W), dtype=NPBF)
    for t, (ti, tj) in enumerate(TAPS):
        for g in range(4):
            xstk[:, 128 * g + 16 * t : 128 * g + 16 * t + 16] = xpb[
                :, 16 * g : 16 * g + 16, ti : ti + H, tj : tj + W
            ]
        gs[:, 16 * t : 16 * t + 16] = gp[:, :, ti : ti + H, tj : tj + W]
        gc[:, 16 * t : 16 * t + 16] = gp[
            :, :, CTR_I : CTR_I + H, CTR_J : CTR_J + W
        ]

    # wstk[g][16*t + i, o] = weight[o, 16g+i, ti, tj]
    wstk = np.zeros((4 * 128, COUT), dtype=np.float32)
    for g in range(4):
        for t, (ti, tj) in enumerate(TAPS):
            wstk[128 * g + 16 * t : 128 * g + 16 * t + 16, :] = weight[
                :, 16 * g : 16 * g + 16, ti, tj
            ].T
    wstk = wstk.astype(NPBF)
    wctr = np.ascontiguousarray(weight[:, :, CTR_I, CTR_J].T)  # (CIN, COUT) fp32

    lhsd = np.zeros((128, 128), dtype=np.float32)
    for t in range(NT):
        lhsd[16 * t : 16 * t + 16, 16 * t : 16 * t + 16] = -0.5
    lhsd = lhsd.astype(NPBF)

    bias2 = bias.reshape(COUT, 1).astype(np.float32)

    in_maps = []
    for i in range(NCORES):
        in_maps.append(
            {
                "xp": np.ascontiguousarray(xp[i]),
                "xstk": np.ascontiguousarray(xstk[i]),
                "gs": np.ascontiguousarray(gs[i]),
                "gc": np.ascontiguousarray(gc[i]),
                "wstk": wstk,
                "wctr": wctr,
                "lhsd": lhsd,
                "bias": bias2,
            }
        )
    return in_maps


def _run(in_maps, trace=False, **kw):
    nc = _get_nc()
    last = None
    for attempt in range(3):
        try:
            res = run_bass_kernel_spmd(
                nc, in_maps, list(range(NCORES)), trace=trace, **kw
            )
            break
        except Exception as e:  # wedged device: wait and retry
            last = e
            import time as _t

            _t.sleep(20 * (attempt + 1))
    else:
        raise last
    out = np.stack([res.results[i]["out"] for i in range(NCORES)], axis=0)
    return out.astype(np.float32), res


def kernel(x, guide, weight, bias):
    in_maps = _prep_inputs(x, guide, weight, bias)
    out, _ = _run(in_maps)
    return out

